# revision 45
# baseline (speedup 1.0000x reference)
"""Data-parallel attention kernel for Trainium2 (8 NeuronCores).

Reference computation (per batch item b):
    scores[q, k] = sum_{hw} query[b, hw, q] * keys[b, hw, k]     (C=256, HW=4096)
    attn = softmax_k(scores)
    out[b, q, hw] = sum_k attn[q, k] * values[b, hw, k]

Sharding: batch axis (B=32) split across 8 cores, 4 items per core, no
cross-core communication.

Design (~108-113us measured, vs the 182us f32-input baseline; rel err
1.23e-3 vs gate 2e-2):
  - Inputs are cast to f16 on the HOST inside kernel(), so the device
    streams 2-byte elements: per-core HBM traffic drops 58.7MB ->
    33.6MB (reads 25.2MB + f16 output 8.4MB).  At the ~358GB/s per-core
    fair share (both cores of each trn2 pair run this kernel) that is a
    ~94us stream + ~6us sequencer boot + ~2.4us final-semaphore drain.
    f16 matmuls run at full PE rate and 11-bit mantissas give BETTER
    accuracy than the old f32r/bf16 mix; bf16 Q/K would hit 1.0e-2.
  - Loads use an hw=(p n) partition mapping and LG=8-chunk load groups
    so each partition line is one 4KB-contiguous DRAM read.  Q7 SWDGE
    descriptor generation paces input issue (~6ns/piece): 512B pieces
    -> 87us of DIRECT2D, 2KB -> 73us, 4KB -> 30us; only the last gets
    generation off the critical path.  Batch 0's first QK load pair is
    split into half-tile DMAs so the first S matmuls wait on 512KB, not
    1MB.  Contractions sum over all hw, so the mapping is free.
  - PE work (~80us busy): S = Q^T K accumulating into one PSUM bank per
    q-block; V PE-transposed ([hw,k]->[k,hw]) via identity matmuls (the
    SDMA XBAR transpose path measured ~41GB/s -- too slow; DVE 32x32
    stream-transpose cannot cross partition banks; gpsimd cannot read
    PSUM); O = A @ V^T f16, N=512 per matmul (PSUM bank cap).
  softmax:  DVE row-max (negated) -> ACT exp(in + bias) with
            accumulated row sums -> DVE reciprocal; normalization is
            folded into the O epilogue, so A stays unnormalized f16.

Scheduling notes (hard-won):
  - All input DMAs ride the single gpsimd SWDGE queue in consumption
    order, V and QK load-groups interleaved 1:1.  HWDGE rings measured
    ~60GB/s on these strided loads (a 512KB ACT-ring group landed at
    22us), so nothing rides them except outputs and the tiny ident.
  - Output DMAs ride the sync HWDGE ring so data-dependent waits never
    block input prefetch.  Outputs are written in [g, c, p] block order
    (host unscrambles), paired two hw-groups per DMA for 2KB pieces;
    the last two groups write singly so the final drain DMA is small.
  - Each V group's transposes are emitted one group ahead of the O
    matmuls (and ahead of the softmax-blocked A^T transposes) so the
    Tensor queue always has ready work.  The Tile scheduler already
    interleaves batch b+1's S matmuls into batch b's O phase on its
    own -- manual cross-batch reordering measured neutral-to-worse.
  - The input stream end (~86us) is SBUF slot-wait paced, not
    bandwidth-paced: deferring all output writes behind a gate (pure-
    read stream) measured WORSE (and cost prefetch depth for SBUF).
    Pool depths: qk 12 / vb 8 load-group tiles; deeper measured worse.
"""

import numpy as np

import concourse.bass as bass
import concourse.tile as tile
from concourse import bacc, mybir
from concourse.bass_utils import run_bass_kernel_spmd
from contextlib import ExitStack

B, H, W, C = 32, 64, 64, 256
N_CORES = 8
B_LOC = B // N_CORES          # 4 batch items per core
HW = H * W                    # 4096
P = 128                       # partitions
N_CHUNK = HW // P             # 32 chunks of 128 hw-rows
SG = 4                        # chunks per S-phase group (512 hw rows)
VG = 4                        # chunks per O-phase group (512 hw rows)
LG = 8                        # chunks per LOAD group (4KB DMA pieces)
N_SGRP = N_CHUNK // SG        # 8
N_VGRP = N_CHUNK // VG        # 8
N_LGRP = N_CHUNK // LG        # 4 load groups per tensor per batch
QB = C // P                   # 2 q-blocks
KC = C // P                   # 2 k-chunks

F32 = mybir.dt.float32
F16 = mybir.dt.float16

_CACHE = {}


def _build():
    nc = bacc.Bacc("TRN2", target_bir_lowering=False, debug=False,
                   num_devices=N_CORES)
    q_ext = nc.dram_tensor("query", [B_LOC, H, W, C], F16,
                           kind="ExternalInput").ap()
    k_ext = nc.dram_tensor("keys", [B_LOC, H, W, C], F16,
                           kind="ExternalInput").ap()
    v_ext = nc.dram_tensor("values", [B_LOC, H, W, C], F16,
                           kind="ExternalInput").ap()
    # Output written in [g, c, p] block order (hw = p*32 + g*VG + c);
    # the host unscrambles. 1KB-contiguous pieces per partition line.
    o_ext = nc.dram_tensor("out", [B_LOC, C, N_VGRP, VG, P], F16,
                           kind="ExternalOutput").ap()

    # [b, hw, c] -> [b, p, n, c] with hw = p*32 + n: each partition line
    # covers consecutive DRAM rows, so a group DMA moves SG*512B = 2KB
    # contiguous pieces (4x fewer SWDGE descriptors than the (n p) split,
    # whose pieces are single 512B c-rows).  The S/O contractions sum
    # over all hw, so the chunk->partition assignment is free.
    qv = q_ext.rearrange("b h w c -> b (h w) c").rearrange(
        "b (p n) c -> b p n c", p=P)
    kv = k_ext.rearrange("b h w c -> b (h w) c").rearrange(
        "b (p n) c -> b p n c", p=P)
    vv = v_ext.rearrange("b h w c -> b (h w) c").rearrange(
        "b (p n) c -> b p n c", p=P)

    with tile.TileContext(nc) as tc, ExitStack() as ctx:
        qk_pool = ctx.enter_context(tc.tile_pool(name="qk", bufs=12))
        vb_pool = ctx.enter_context(tc.tile_pool(name="vb", bufs=8))
        vt_pool = ctx.enter_context(tc.tile_pool(name="vt", bufs=8))
        a_pool = ctx.enter_context(tc.tile_pool(name="a", bufs=3))
        at_pool = ctx.enter_context(tc.tile_pool(name="at", bufs=4))
        o_pool = ctx.enter_context(tc.tile_pool(name="o", bufs=6))
        stat_pool = ctx.enter_context(tc.tile_pool(name="stat", bufs=2 * B_LOC))
        singles = ctx.enter_context(tc.tile_pool(name="singles", bufs=1))
        ps_s = ctx.enter_context(tc.tile_pool(name="ps_s", bufs=2, space="PSUM"))
        ps_vt = ctx.enter_context(tc.tile_pool(name="ps_vt", bufs=3, space="PSUM"))
        ps_o = ctx.enter_context(tc.tile_pool(name="ps_o", bufs=3, space="PSUM"))

        # Identity for PE transposes, embedded in the NEFF as a Const
        # DRAM tensor (loaded at model-load time, not exec time).
        ident_dram = nc.inline_tensor(
            np.eye(P, dtype=np.float16), name="ident_const")
        ident = singles.tile([P, P], F16)

        def issue_qk_group(b, g):
            # 1MB load pair in 4KB-contiguous pieces: descriptor
            # generation on the Q7 (~6ns/piece) was pacing the input
            # stream at 512KB/2KB granularity (73us of DIRECT2D).
            q_t = qk_pool.tile([P, LG, C], F16, tag="q", name=f"q_t_{b}_{g}")
            nc.gpsimd.dma_start(out=q_t[:],
                                in_=qv[b, :, g * LG:(g + 1) * LG, :])
            k_t = qk_pool.tile([P, LG, C], F16, tag="k", name=f"k_t_{b}_{g}")
            nc.gpsimd.dma_start(out=k_t[:],
                                in_=kv[b, :, g * LG:(g + 1) * LG, :])
            return (q_t, k_t)

        def issue_v_group(b, g):
            vb_t = vb_pool.tile([P, LG, C], F16, tag="vb",
                                name=f"vb_t_{b}_{g}")
            nc.gpsimd.dma_start(out=vb_t[:],
                                in_=vv[b, :, g * LG:(g + 1) * LG, :])
            return vb_t

        # Input DMAs ride the single gpsimd SWDGE queue (program order);
        # issue in consumption order.  (HWDGE rings measured ~60GB/s on
        # these strided loads -- a 512KB ACT-ring group landed at 22us --
        # so everything stays on SWDGE.)
        def issue_qk_group_split(b, g):
            # Same tile, two half-DMAs: the first S matmuls depend only
            # on the first half, starting the PE ~2us earlier.
            q_t = qk_pool.tile([P, LG, C], F16, tag="q", name=f"q_t_{b}_{g}")
            k_t = qk_pool.tile([P, LG, C], F16, tag="k", name=f"k_t_{b}_{g}")
            for h in range(2):
                sl = slice(g * LG + h * SG, g * LG + (h + 1) * SG)
                nc.gpsimd.dma_start(out=q_t[:, h * SG:(h + 1) * SG, :],
                                    in_=qv[b, :, sl, :])
                nc.gpsimd.dma_start(out=k_t[:, h * SG:(h + 1) * SG, :],
                                    in_=kv[b, :, sl, :])
            return (q_t, k_t)

        qk_by_batch = {0: [issue_qk_group_split(0, g) if g == 0 else
                           issue_qk_group(0, g) for g in range(N_LGRP)]}
        nc.sync.dma_start(out=ident[:], in_=ident_dram.ap())

        qk_flat = [(bb, g) for bb in range(1, B_LOC) for g in range(N_LGRP)]
        qi = 0

        for b in range(B_LOC):
            # Interleaved input issue for this phase.
            vload_tiles = []
            for g in range(N_LGRP):
                vload_tiles.append(issue_v_group(b, g))
                if qi < len(qk_flat):
                    bb, gg = qk_flat[qi]
                    qi += 1
                    qk_by_batch.setdefault(bb, []).append(
                        issue_qk_group(bb, gg))

            # ---- S = Q^T K (f16), accumulate over hw ----
            s_ps = [ps_s.tile([P, C], F32, tag="ps_s", name=f"s_ps_{b}_{qb}")
                    for qb in range(QB)]
            for g in range(N_LGRP):
                q_t, k_t = qk_by_batch[b][g]
                for c in range(LG):
                    for qb in range(QB):
                        nc.tensor.matmul(
                            s_ps[qb][:],
                            lhsT=q_t[:, c, qb * P:(qb + 1) * P],
                            rhs=k_t[:, c, :],
                            start=(g == 0 and c == 0),
                            stop=(g == N_LGRP - 1 and c == LG - 1),
                        )

            # ---- softmax over k (free axis) ----
            negmax = stat_pool.tile([P, QB, 1], F32, tag="negmax")
            rowsum = stat_pool.tile([P, QB, 1], F32, tag="rowsum")
            recip = stat_pool.tile([P, QB, 1], F32, tag="recip")
            a_sb = a_pool.tile([P, QB, C], F16, tag="a")
            for qb in range(QB):
                nc.vector.tensor_reduce(
                    out=negmax[:, qb, :], in_=s_ps[qb][:],
                    axis=mybir.AxisListType.X, op=mybir.AluOpType.max,
                    negate=True)
                nc.scalar.activation(
                    out=a_sb[:, qb, :], in_=s_ps[qb][:],
                    func=mybir.ActivationFunctionType.Exp,
                    bias=negmax[:, qb, :], scale=1.0,
                    accum_out=rowsum[:, qb, :])
                nc.vector.reciprocal(out=recip[:, qb, :], in_=rowsum[:, qb, :])

            # ---- V^T via PE transposes, pipelined one group ahead ----
            def vt_group(g):
                vb_t = vload_tiles[g // 2]
                off = (g % 2) * VG
                vt_ps = ps_vt.tile([P, KC, VG, P], F16, tag="ps_vt")
                for c in range(VG):
                    for kc in range(KC):
                        nc.tensor.transpose(
                            out=vt_ps[:, kc, c, :],
                            in_=vb_t[:, off + c, kc * P:(kc + 1) * P],
                            identity=ident[:])
                vt_sb = vt_pool.tile([P, KC, VG, P], F16, tag="vt")
                # Alternate copy engine so this stage never stacks up on
                # one engine.  (gpsimd can't read PSUM, so it can't help.)
                if g % 2 == 0:
                    nc.vector.tensor_copy(out=vt_sb[:], in_=vt_ps[:])
                else:
                    nc.scalar.copy(out=vt_sb[:], in_=vt_ps[:])
                return vt_sb

            # Group 0's V-transposes are emitted BEFORE the A^T
            # transposes: A^T waits on the softmax exp, and the in-order
            # Tensor queue would otherwise idle the PE during that wait.
            vt_cur = vt_group(0)

            # ---- A^T via PE transposes: at[:, kc, qb, :] = A[qb-block, kc-chunk]^T
            # at_ps borrows a ps_o slot (not ps_s): sharing ps_s with the
            # S accumulators made batch b+1's second S tile wait for
            # batch b's A^T copy, stalling the scheduler's cross-batch
            # S/O interleave by ~1us per batch.
            at_ps = ps_o.tile([P, KC, QB, P], F16, tag="ps_o")
            for kc in range(KC):
                for qb in range(QB):
                    nc.tensor.transpose(
                        out=at_ps[:, kc, qb, :],
                        in_=a_sb[:, qb, kc * P:(kc + 1) * P],
                        identity=ident[:])
            at_sb = at_pool.tile([P, KC, QB, P], F16, tag="at")
            nc.vector.tensor_copy(out=at_sb[:], in_=at_ps[:])

            # ---- O = A @ V^T, f16, streamed over hw groups ----
            for g in range(N_VGRP):
                vt_sb = vt_cur
                # Emit next group's transposes ahead of this group's
                # matmuls so the PE always has transpose work queued
                # while epilogue/copy stages drain.
                if g + 1 < N_VGRP:
                    vt_cur = vt_group(g + 1)
                paired = g < N_VGRP - 2
                if g % 2 == 0:
                    # Pair two groups per output tile so each output DMA
                    # writes 2KB-contiguous pieces per partition line.
                    # The last two groups write singly so the final
                    # drain DMA is half-size.
                    o_sbs = [o_pool.tile([P, 2, VG * P], F16, tag=f"o{qb}",
                                          name=f"o_sb_{b}_{g}_{qb}")
                             for qb in range(QB)]
                for qb in range(QB):
                    o_ps = ps_o.tile([P, VG * P], F32, tag="ps_o")
                    for kc in range(KC):
                        nc.tensor.matmul(
                            o_ps[:],
                            lhsT=at_sb[:, kc, qb, :],
                            rhs=vt_sb[:, kc, :, :].rearrange("p c x -> p (c x)"),
                            start=(kc == 0), stop=(kc == KC - 1),
                        )
                    # Split epilogues between ACT and DVE to balance load.
                    if qb == 0:
                        nc.scalar.activation(
                            out=o_sbs[qb][:, g % 2, :], in_=o_ps[:],
                            func=mybir.ActivationFunctionType.Copy,
                            scale=recip[:, qb, :])
                    else:
                        nc.vector.tensor_scalar_mul(
                            o_sbs[qb][:, g % 2, :], o_ps[:], recip[:, qb, :])
                    if paired and g % 2 == 1:
                        nc.sync.dma_start(
                            out=o_ext[b, qb * P:(qb + 1) * P, g - 1:g + 1, :, :],
                            in_=o_sbs[qb][:].rearrange(
                                "q t (c p) -> q t c p", p=P))
                    elif not paired:
                        nc.sync.dma_start(
                            out=o_ext[b, qb * P:(qb + 1) * P, g, :, :],
                            in_=o_sbs[qb][:, g % 2, :].rearrange(
                                "q (c p) -> q c p", p=P))

    nc.compile()
    return nc


def _get_nc():
    if "nc" not in _CACHE:
        _CACHE["nc"] = _build()
    return _CACHE["nc"]


def prep_in_maps(query, keys, values):
    """Host-side prep: cast f32 -> f16 and slice the batch across cores."""
    q16 = np.ascontiguousarray(np.asarray(query)).astype(np.float16)
    k16 = np.ascontiguousarray(np.asarray(keys)).astype(np.float16)
    v16 = np.ascontiguousarray(np.asarray(values)).astype(np.float16)
    in_maps = []
    for i in range(N_CORES):
        sl = slice(i * B_LOC, (i + 1) * B_LOC)
        in_maps.append({
            "query": np.ascontiguousarray(q16[sl]),
            "keys": np.ascontiguousarray(k16[sl]),
            "values": np.ascontiguousarray(v16[sl]),
        })
    return in_maps


def assemble_out(res):
    """Host-side postprocess: gather per-core f16 outputs, unscramble the
    hw axis (written as [g, c, p] blocks; hw = p*32 + g*VG + c), -> f32."""
    parts = []
    for i in range(N_CORES):
        arr = res.results[i]["out"]          # [B_LOC, C, N_VGRP, VG, P]
        arr = arr.transpose(0, 1, 4, 2, 3).reshape(B_LOC, C, H, W)
        parts.append(arr.astype(np.float32))
    return np.concatenate(parts, axis=0)


def kernel(query, keys, values):
    assert np.asarray(query).shape == (B, H, W, C)
    nc = _get_nc()
    in_maps = prep_in_maps(query, keys, values)
    res = run_bass_kernel_spmd(nc, in_maps, core_ids=list(range(N_CORES)))
    return assemble_out(res)
